# revision 18
# baseline (speedup 1.0000x reference)
"""Trainium2 Bass kernel for the memristive-crossbar linear layer.

Reference computation (see problem statement):
    Wt   = weight.T                                  [in=1024, out=1024]
    G    = quantize(weight_mapping(Wt))              (affine map, 4-bit snap)
    Geff = 1/(1/G + r_series)                        (Jeong IR-drop model)
    currents       = x @ Geff
    ideal_currents = x @ G
    corr   = currents.mean(1) / ideal_currents.mean(1)
    output = (currents - b*x.sum(1, keepdims=True)) / a + bias * corr[:, None]

Restructuring (same algebra as the previous 52us fp16 version):
    (currents - b*sx)/a  ==  x @ M     with M = (Geff - b)/a
    currents.mean(1)     ==  x @ u     with u = Geff.mean(axis=1)
    ideal_currents.mean(1)== x @ v     with v = G.mean(axis=1)

Everything except the single dense matmul is off-chip:
  - M, u, v are weight-derived -> host.
  - corr = (x@u)/(x@v) is 34 MFLOP (0.2% of the 17 GFLOP matmul) -> host.
  - M is split as M = mbar[None,:] + M0 (column means removed). The chip
    computes Y0 = x @ M0 only; the host adds back the two rank-1 terms
    sx[:,None]*mbar[None,:] + bias[None,:]*corr[:,None] (sx = x.sum(1)).
    Removing the large systematic IR-drop component shrinks |Y0| to ~4,
    which lets BOTH M0 and the output live in fp8 e4m3 (measured
    absmax-rel error 1.8e-3 vs the fp32 reference; gate is 2e-2).

The chip work per core (batch-sharded 8 ways, 1024 rows/core):
    Y0[1024,1024] = x_shard[1024,1024] @ M0[1024,1024]   (fp8 in, fp8 out)

PE runs DoubleRow perf mode: each matmul instruction consumes TWO
128-deep k-tiles (stationary x slice [128,2,128], moving M0 slice
[128,2,512]) at ~216ns -> 155 TF/s effective, the fp8 peak. 64 matmuls
= 13.8us PE floor. DMA: 1 MB x + 1 MB M0 in, 1 MB Y0 out = ~8.4us at
the 358 GB/s HBM-per-core limit -> PE-bound (the "ridge").

Schedule: k-group 0 is split into separate half tiles so the first
matmuls start as soon as ~128KB has landed; x on the sync HWDGE ring,
M0 on the scalar ring; junk matmuls flip the HAM clock gate during the
DMA window; a 4-batch-tile chase (all 8 PSUM banks) consumes k-groups
as they land; the remaining 4 tiles stream back-to-back while per-half
PSUM->SBUF fp8 casts (DVE for h0, ACT for h1) and per-tile stores
(sync ring) drain behind the PE; the last tile splits its stores
across both rings to shorten the tail.
"""

import numpy as np
import ml_dtypes

import concourse.bacc as bacc
import concourse.bass as bass
import concourse.mybir as mybir
import concourse.tile as tile
from concourse.bass_utils import run_bass_kernel_spmd

# ---- problem constants (hardcoded; must match the module init kwargs) ----
R_HRS = 1000000.0
R_LRS = 1000.0
PARASITIC_R = 2.0
BITS = 4
BATCH, IN_F, OUT_F = 8192, 1024, 1024

N_CORES = 8
B_LOC = BATCH // N_CORES          # rows of x per core
BT = B_LOC // 128                 # batch tiles per core

# "fp8dr": e4m3 in/out, DoubleRow PE (2 k-tiles per matmul). "fp16": plain.
MM_MODE = "fp8dr"

WARM = 8                          # junk matmuls to flip the HAM clock gate
CHASE = 4                         # batch tiles chasing the input DMAs

_F32 = mybir.dt.float32
_F16 = mybir.dt.float16


def _mode_params(mm_mode):
    if mm_mode == "fp8dr":
        return dict(
            dt=mybir.dt.float8e4,
            np_dt=ml_dtypes.float8_e4m3,
            out_dt=mybir.dt.float8e4,
            np_out=ml_dtypes.float8_e4m3,
            kstep=2,
        )
    if mm_mode == "fp16":
        return dict(
            dt=_F16, np_dt=np.float16, out_dt=_F16, np_out=np.float16, kstep=1
        )
    raise ValueError(mm_mode)


def _prepare_weights(weight: np.ndarray):
    """Host-side weight preprocessing -> (M [IN_F,OUT_F] f32, u, v [IN_F] f32).

    Follows the reference op-for-op in fp32 (scalars kept in double and
    rounded at use, matching jax weak-typed scalar promotion).
    """
    G_hrs = 1.0 / R_HRS
    G_lrs = 1.0 / R_LRS
    Wt = np.ascontiguousarray(weight.T.astype(np.float32, copy=False))
    Wmin = Wt.min()
    Wmax = Wt.max()
    G = (Wt - Wmin) / (Wmax - Wmin) * np.float32(G_lrs - G_hrs) + np.float32(G_hrs)
    step = (G_lrs - G_hrs) / (2**BITS - 1)
    G = np.round((G - np.float32(G_hrs)) / np.float32(step)) * np.float32(step) + np.float32(
        G_hrs
    )
    rows, cols = G.shape
    r_series = np.float32(PARASITIC_R) * (
        (np.arange(cols, dtype=np.float32) + np.float32(1.0))[None, :]
        + (np.float32(rows) - np.arange(rows, dtype=np.float32))[:, None]
    )
    G_eff = np.float32(1.0) / (np.float32(1.0) / G + r_series)
    a = np.float32(G_lrs - G_hrs) / (Wmax - Wmin)
    b = np.float32(G_hrs) - a * Wmin
    M = (G_eff - b) / a
    u = G_eff.mean(axis=1, dtype=np.float32)
    v = G.mean(axis=1, dtype=np.float32)
    return M.astype(np.float32), u, v


def _interleave_k(arr_kx, kstep):
    """[K, N] -> [K//(128*kstep), 128, kstep, N] with k = p*(128*kstep) + t*128 + q."""
    K, N = arr_kx.shape
    kp = K // (128 * kstep)
    return np.ascontiguousarray(
        arr_kx.reshape(kp, kstep, 128, N).transpose(0, 2, 1, 3)
    )


def _build(mm_mode: str, warm_memset: bool):
    """Build the per-core Bass program (identical on all 8 cores)."""
    prm = _mode_params(mm_mode)
    mm_dt, out_dt, kstep = prm["dt"], prm["out_dt"], prm["kstep"]
    kp_n = IN_F // (128 * kstep)  # k-groups (4 for fp8dr, 8 for fp16)
    perf_mode = mybir.MatmulPerfMode.DoubleRow if kstep == 2 else None

    nc = bacc.Bacc(
        "TRN2", target_bir_lowering=False, debug=False, enable_partition_id=False
    )

    xt_d = nc.dram_tensor("xt", (kp_n, 128, kstep, B_LOC), mm_dt, kind="ExternalInput")
    m_d = nc.dram_tensor("mext", (kp_n, 128, kstep, OUT_F), mm_dt, kind="ExternalInput")
    out_d = nc.dram_tensor("out", (BT, 128, OUT_F), out_dt, kind="ExternalOutput")

    xt_t = xt_d.ap().rearrange("kp p t b -> p kp t b")   # [128, kp, kstep, B_LOC]
    m_t = m_d.ap().rearrange("kp p t c -> p kp t c")     # [128, kp, kstep, OUT_F]
    out_t = out_d.ap()                                   # [BT, 128, OUT_F]

    with tile.TileContext(nc) as tc:
        with (
            tc.tile_pool(name="big", bufs=1) as big,
            tc.tile_pool(name="psum", bufs=4, space="PSUM") as psum,
        ):
            # warm-up input for the HAM-flipping junk matmuls. Reading an
            # uninitialized tile is fine (results are discarded by the real
            # start=True groups); the optional memset only exists to satisfy
            # use-before-def checking if the framework enforces it.
            warm_in = big.tile([128, 512], mm_dt)
            if warm_memset:
                nc.gpsimd.memset(warm_in, 0.0)

            # one tile per k-group so matmul deps are per-DMA; x on the sync
            # HWDGE ring, M on the scalar ring (256KB contiguous transfers)
            x_sb = {}
            m_sb = {}
            for p in range(kp_n):
                x_sb[p] = big.tile([128, kstep, B_LOC], mm_dt, name=f"x{p}")
                m_sb[p] = big.tile([128, kstep, OUT_F], mm_dt, name=f"m{p}")
            for p in range(kp_n):
                nc.sync.dma_start(out=x_sb[p], in_=xt_t[:, p])
                nc.scalar.dma_start(out=m_sb[p], in_=m_t[:, p])

            def x_slice(p, bt):
                t = x_sb[p]
                c = bt * 128
                return t[:, :, c : c + 128] if kstep == 2 else t[:, 0, c : c + 128]

            def m_slice(p, h):
                t = m_sb[p]
                c = h * 512
                return t[:, :, c : c + 512] if kstep == 2 else t[:, 0, c : c + 512]

            # output staging tiles (fp8), all resident -- no recycling stalls
            o_sb = [big.tile([128, OUT_F], out_dt, name=f"o{bt}") for bt in range(BT)]

            # PSUM: one [128,512] half-tile per (bt, h) accumulation group --
            # 8 banks = 8 live halves; dep tracking is tile-granular, so
            # per-half tiles let each cast start right after its own stop and
            # free its bank for the stream without waiting on the sibling half
            def ps_pair(bt):
                return [
                    psum.tile([128, 512], _F32, tag="ps", name=f"ps{bt}h{h}")
                    for h in (0, 1)
                ]

            ps_tiles = {bt: ps_pair(bt) for bt in range(CHASE)}

            # junk matmuls into ps0h0 (cleared later by the real start=True
            # group): keeps the PE busy through the HAM SHORT window
            for _ in range(WARM):
                nc.tensor.matmul(ps_tiles[0][0], warm_in[:, 0:128], warm_in)

            def mm(bt, ps_t, p, h):
                nc.tensor.matmul(
                    ps_t[h],
                    x_slice(p, bt),
                    m_slice(p, h),
                    start=(p == 0),
                    stop=(p == kp_n - 1),
                    perf_mode=perf_mode,
                )

            # chase: consume each k-group as it lands; bt-major so each chase
            # tile's halves stop early in the final wave and their casts can
            # free the PSUM banks before the stream needs them
            for p in range(kp_n):
                for bt in range(CHASE):
                    for h in (0, 1):
                        mm(bt, ps_tiles[bt], p, h)

            def copies(bt, ps_t):
                # per-half PSUM -> SBUF fp8 casts: h0 on DVE, h1 on ACT
                nc.vector.tensor_copy(out=o_sb[bt][:, 0:512], in_=ps_t[0])
                nc.scalar.copy(out=o_sb[bt][:, 512:1024], in_=ps_t[1])

            def store(bt):
                eng = nc.sync if bt % 2 == 0 else nc.scalar
                eng.dma_start(out=out_t[bt], in_=o_sb[bt])

            # chase epilogues (their PSUMs complete first; stream tiles
            # recycle the 8-half-buffer PSUM pool behind them)
            for bt in range(CHASE):
                copies(bt, ps_tiles.pop(bt))
                store(bt)

            for bt in range(CHASE, BT):
                ps_t = ps_pair(bt)
                if bt == BT - 1:
                    # h-outer: h0 stops 4 matmuls early; its DVE cast runs
                    # concurrently with the final h1 matmuls -> shorter tail
                    for h in (0, 1):
                        for p in range(kp_n):
                            mm(bt, ps_t, p, h)
                else:
                    for p in range(kp_n):
                        for h in (0, 1):
                            mm(bt, ps_t, p, h)
                copies(bt, ps_t)
                store(bt)

    nc.compile()
    return nc


_NC_CACHE: dict[str, object] = {}


def _get_nc(mm_mode: str):
    if mm_mode not in _NC_CACHE:
        try:
            _NC_CACHE[mm_mode] = _build(mm_mode, warm_memset=False)
        except Exception:
            _NC_CACHE[mm_mode] = _build(mm_mode, warm_memset=True)
    return _NC_CACHE[mm_mode]


def make_in_maps(x, weight, bias, mm_mode=None):
    """Host-side sharding: per-core input dicts + host epilogue terms."""
    mm_mode = mm_mode or MM_MODE
    prm = _mode_params(mm_mode)
    np_dt, kstep = prm["np_dt"], prm["kstep"]
    x = np.asarray(x, dtype=np.float32)
    weight = np.asarray(weight, dtype=np.float32)
    bias = np.asarray(bias, dtype=np.float32)
    M, u, v = _prepare_weights(weight)
    mbar = M.mean(axis=0)                     # [OUT_F] column means
    M0 = M - mbar[None, :]
    m_il = _interleave_k(M0.astype(np_dt), kstep)
    corr = (x @ u) / (x @ v)                  # [BATCH]
    sx = x.sum(axis=1)                        # [BATCH]
    in_maps = []
    for c in range(N_CORES):
        xs = x[c * B_LOC : (c + 1) * B_LOC]
        xT8 = _interleave_k(np.ascontiguousarray(xs.T).astype(np_dt), kstep)
        in_maps.append({"xt": xT8, "mext": m_il})
    return in_maps, corr, sx, mbar, bias


def kernel(x, weight, bias, mm_mode=None, trace=False):
    mm_mode = mm_mode or MM_MODE
    nc = _get_nc(mm_mode)
    in_maps, corr, sx, mbar, bias_f = make_in_maps(x, weight, bias, mm_mode)
    res = run_bass_kernel_spmd(
        nc, in_maps, core_ids=list(range(N_CORES)), trace=trace
    )
    y = np.concatenate(
        [res.results[c]["out"].reshape(B_LOC, OUT_F) for c in range(N_CORES)], axis=0
    )
    out = y.astype(np.float32)
    out += sx[:, None] * mbar[None, :]
    out += bias_f[None, :] * corr[:, None]
    if trace:
        return out, res
    return out
